# revision 1
# baseline (speedup 1.0000x reference)
"""MultiHeadCrossAttention kernel for 8 Trainium2 NeuronCores.

Sharding: pure data-parallel over batch (B=8 -> 1 batch element per core).
Per-core layout strategy:
  - Activations transposed on-chip via PE transpose -> feature-major xT/keyT/valueT.
  - Projections produce qT,kT feature-major [E, L] and v token-major [L, E]
    (v stored with a ones-column per head for the softmax denominator).
  - Attention per head in transposed orientation: scoresT[k,q] = kT_h^T-slices,
    exp on ScalarE (no max subtraction: |scores*0.125| < ~4), attn_unnormT and
    denominator from one matmul using the [v_h | 1] stationary operand.
  - attn_weights (mean over heads of normalized probs) accumulated in PSUM via
    identity matmuls, transposed back to natural [q,k] at the end of each
    q-block with PE transposes.
  - out_proj + residual + LayerNorm fused per q-block of 256 rows.
All matmuls run as float32r (full PE rate at free-dim >= 256).
"""

import numpy as np
from contextlib import ExitStack

import concourse.bacc as bacc
import concourse.bass as bass
import concourse.tile as tile
from concourse import mybir
from concourse.bass_utils import run_bass_kernel_spmd
from concourse.masks import make_identity

E = 1024
H = 16
DH = 64
L = 1024
P = 128
QB = 256          # q-block size
NQB = L // QB     # 4
NKT = L // P      # 8 k-tiles
NEC = E // P      # 8 feature chunks
VS = H * (DH + 1)  # 1040 v columns per k-chunk (65 per head)
LN_EPS = 1e-5

F32 = mybir.dt.float32
F32R = mybir.dt.float32r
AF = mybir.ActivationFunctionType
OP = mybir.AluOpType


def _emit(nc, tc, io):
    x_q, k_in, v_in = io["x_q"], io["k_in"], io["v_in"]
    wqT, wkT, wvT, woT_d = io["wqT"], io["wkT"], io["wvT"], io["woT"]
    b_all, gb = io["b_all"], io["gb"]
    y_out, w_out = io["y_out"], io["w_out"]

    ctx = tc.ctx  # ExitStack from caller
    ctx.enter_context(nc.allow_low_precision("fp32r tiles"))

    const = ctx.enter_context(tc.tile_pool(name="const", bufs=1))
    persist = ctx.enter_context(tc.tile_pool(name="persist", bufs=1))
    psum_acc = ctx.enter_context(tc.tile_pool(name="psum_acc", bufs=4, space="PSUM"))
    psum_sc = ctx.enter_context(tc.tile_pool(name="psum_sc", bufs=2, space="PSUM"))
    psum_av = ctx.enter_context(tc.tile_pool(name="psum_av", bufs=2, space="PSUM"))

    ident_f = const.tile([P, P], F32)
    make_identity(nc, ident_f[:])
    ident = const.tile([P, P], F32R)
    nc.vector.tensor_copy(ident[:], ident_f[:])
    ones1f = const.tile([1, P], F32)
    nc.vector.memset(ones1f[:], 1.0)
    ones1 = const.tile([1, P], F32R)
    nc.vector.tensor_copy(ones1[:], ones1f[:])
    onesP = const.tile([P, 1], F32)
    nc.vector.memset(onesP[:], 1.0)
    eps_sb = const.tile([P, 1], F32)
    nc.vector.memset(eps_sb[:], LN_EPS)

    # biases: b_all DRAM [4, 1024] rows = bq, bk, bv, bo ; gb DRAM [2, 1024] = gamma, beta
    bqk_col = const.tile([P, 2 * NEC], F32)  # [:,0:8]=bq cols, [:,8:16]=bk cols
    for i in range(2):
        nc.sync.dma_start(
            out=bqk_col[:, NEC * i:NEC * (i + 1)],
            in_=b_all[i, :].rearrange("(m p) -> p m", p=P).bitcast(F32),
        )
    bvbo_row = const.tile([1, 2 * E], F32R)  # [0:1024]=bv, [1024:2048]=bo
    nc.sync.dma_start(out=bvbo_row[:, 0:E], in_=b_all[2:3, :])
    nc.sync.dma_start(out=bvbo_row[:, E:2 * E], in_=b_all[3:4, :])
    gb_row = const.tile([1, 2 * E], F32R)
    nc.sync.dma_start(out=gb_row[:], in_=gb[:])

    qT = persist.tile([P, NEC * L], F32R)      # [e_out, l] chunks of 128 rows
    kT = persist.tile([P, NEC * L], F32R)
    v_sb = persist.tile([P, NKT * VS], F32R)   # token-major v, 65-wide head slots
    op_ = onesP[:]
    nc.vector.tensor_copy(
        out=v_sb[:].rearrange("p (n d) -> p n d", d=DH + 1)[:, :, DH:DH + 1],
        in_=bass.AP(tensor=op_.tensor, offset=op_.offset,
                    ap=[op_.ap[0], [0, H * NKT], [0, 1]]),
    )

    # ---------------- phase 1: transposes + projections ----------------
    with tc.tile_pool(name="wt", bufs=1) as wt_pool, \
         tc.tile_pool(name="ld", bufs=3) as ld_pool, \
         tc.tile_pool(name="actT", bufs=1) as actT_pool:

        for ti, (src, w_d) in enumerate([(x_q, wqT), (k_in, wkT), (v_in, wvT)]):
            # transposed activation aT [e_in, l]
            aT = actT_pool.tile([P, NEC * L], F32R, tag="actT")
            for lc in range(NKT):
                nat = ld_pool.tile([P, E], F32R, tag="ld")
                nc.sync.dma_start(out=nat[:], in_=src[P * lc:P * (lc + 1), :])
                for ep in range(NEC // 2):
                    tp = psum_av.tile([P, 2 * P], F32R, tag="av", name=f"tp_{ti}_{lc}_{ep}")
                    for sub in range(2):
                        ec = 2 * ep + sub
                        nc.tensor.transpose(
                            tp[:, P * sub:P * (sub + 1)],
                            nat[:, P * ec:P * (ec + 1)], ident[:],
                        )
                    for sub in range(2):
                        ec = 2 * ep + sub
                        dst = aT[:, L * ec + P * lc: L * ec + P * lc + P]
                        if (lc + ep) % 2 == 0:
                            nc.scalar.copy(dst, tp[:, P * sub:P * (sub + 1)])
                        else:
                            nc.vector.tensor_copy(dst, tp[:, P * sub:P * (sub + 1)])
            wt = wt_pool.tile([P, NEC * E], F32R, tag="wt")
            for c in range(NEC):
                nc.sync.dma_start(
                    out=wt[:, E * c:E * (c + 1)], in_=w_d[P * c:P * (c + 1), :]
                )
            tiles16 = [(m, n) for m in range(NEC) for n in range(2)]
            for g in range(0, 16, 4):
                grp = tiles16[g:g + 4]
                psums = [
                    psum_acc.tile([P, 512], F32, tag="acc", name=f"ps_{ti}_{g}_{i}")
                    for i in range(len(grp))
                ]
                for c in range(NEC):
                    for i, (m, n) in enumerate(grp):
                        if ti < 2:  # qT / kT : feature-major out
                            lhsT = wt[:, E * c + P * m: E * c + P * (m + 1)]
                            rhs = aT[:, L * c + 512 * n: L * c + 512 * (n + 1)]
                        else:       # v natural
                            lhsT = aT[:, L * c + P * m: L * c + P * (m + 1)]
                            rhs = wt[:, E * c + 512 * n: E * c + 512 * (n + 1)]
                        nc.tensor.matmul(
                            psums[i][:], lhsT, rhs,
                            start=(c == 0), stop=(c == NEC - 1 and ti < 2),
                        )
                for i, (m, n) in enumerate(grp):
                    if ti < 2:
                        dst = (qT if ti == 0 else kT)[:, L * m + 512 * n: L * m + 512 * (n + 1)]
                        nc.vector.tensor_scalar_add(
                            out=dst, in0=psums[i][:],
                            scalar1=bqk_col[:, NEC * ti + m: NEC * ti + m + 1],
                        )
                    else:
                        # bias via ones-row matmul, then strided evict into head slots
                        nc.tensor.matmul(
                            psums[i][:], ones1[0:1, :],
                            bvbo_row[0:1, 512 * n:512 * (n + 1)],
                            start=False, stop=True,
                        )
                        dst = v_sb[:, VS * m + 520 * n: VS * m + 520 * (n + 1)]
                        nc.vector.tensor_copy(
                            out=dst.rearrange("p (h d) -> p h d", d=DH + 1)[:, :, 0:DH],
                            in_=psums[i][:].rearrange("p (h d) -> p h d", d=DH),
                        )

    # ---------------- phase 2: attention + out_proj + LN ----------------
    with tc.tile_pool(name="wo", bufs=1) as wo_pool, \
         tc.tile_pool(name="expT", bufs=2) as expT_pool, \
         tc.tile_pool(name="attnT", bufs=1) as attnT_pool, \
         tc.tile_pool(name="invbc", bufs=2) as invbc_pool, \
         tc.tile_pool(name="accq", bufs=1) as accq_pool, \
         tc.tile_pool(name="wnat", bufs=4) as wnat_pool, \
         tc.tile_pool(name="xqb", bufs=1) as xqb_pool, \
         tc.tile_pool(name="ysb", bufs=1) as ysb_pool, \
         tc.tile_pool(name="small", bufs=2) as small:

        woT = wo_pool.tile([P, NEC * E], F32R, tag="wo")
        for c in range(NEC):
            nc.sync.dma_start(out=woT[:, E * c:E * (c + 1)], in_=woT_d[P * c:P * (c + 1), :])
        gamma_bc = wo_pool.tile([P, E], mybir.dt.bfloat16, tag="gbc")
        beta_bc = wo_pool.tile([P, E], mybir.dt.bfloat16, tag="bbc")
        for i, dstt in enumerate([gamma_bc, beta_bc]):
            for hf in range(2):
                bcp = psum_sc.tile([P, 512], F32, tag="sc")
                nc.tensor.matmul(
                    bcp[:], ones1[0:1, :],
                    gb_row[0:1, E * i + 512 * hf: E * i + 512 * (hf + 1)],
                    start=True, stop=True,
                )
                nc.scalar.copy(dstt[:, 512 * hf:512 * (hf + 1)], bcp[:])

        for qb in range(NQB):
            q0 = QB * qb
            attnT = attnT_pool.tile([P, NEC * QB], F32R, tag="attnT")
            accs = [
                psum_acc.tile([P, 512], F32, tag="acc", name=f"acc_{qb}_{j}")
                for j in range(4)
            ]
            def head_front(h):
                hb = (h % 2) * DH
                hc = h // 2
                expT = expT_pool.tile(
                    [P, NKT * QB], F32R, tag="expT", name=f"expT_{qb}_{h}"
                )
                for j in range(4):
                    sc = psum_sc.tile([P, 512], F32, tag="sc", name=f"sc_{qb}_{h}_{j}")
                    for half in range(2):
                        kt = 2 * j + half
                        lhsT = kT[hb:hb + DH, L * hc + P * kt: L * hc + P * (kt + 1)]
                        rhs = qT[hb:hb + DH, L * hc + q0: L * hc + q0 + QB]
                        nc.tensor.matmul(
                            sc[:, QB * half:QB * (half + 1)],
                            lhsT, rhs,
                            start=True, stop=True,
                        )
                    nc.scalar.activation(
                        expT[:, 512 * j:512 * (j + 1)], sc[:], AF.Exp, scale=0.125
                    )
                return expT

            def head_tail(h, expT):
                hb = (h % 2) * DH
                hc = h // 2
                av = psum_av.tile([DH + 1, QB], F32, tag="av", name=f"av_{qb}_{h}")
                for kt in range(NKT):
                    nc.tensor.matmul(
                        av[:],
                        v_sb[:, VS * kt + (DH + 1) * h: VS * kt + (DH + 1) * (h + 1)],
                        expT[:, QB * kt:QB * (kt + 1)],
                        start=(kt == 0), stop=(kt == NKT - 1),
                    )
                inv = small.tile([1, QB], F32R, tag="inv", name=f"inv_{qb}_{h}")
                nc.vector.reciprocal(inv[:], av[DH:DH + 1, :])
                bcp = psum_sc.tile([P, QB], F32, tag="sc", name=f"bcp_{qb}_{h}")
                nc.tensor.matmul(
                    bcp[:], ones1[0:1, :], inv[:],
                    start=True, stop=True,
                )
                inv_bc = invbc_pool.tile([P, QB], F32, tag="invbc", name=f"ib_{qb}_{h}")
                nc.scalar.copy(inv_bc[:], bcp[:])
                nc.vector.tensor_tensor(
                    out=attnT[hb:hb + DH, QB * hc:QB * (hc + 1)],
                    in0=av[0:DH, :], in1=inv_bc[0:DH, :], op=OP.mult,
                )
                iap = inv_bc[:]
                bc_ap = bass.AP(
                    tensor=iap.tensor, offset=iap.offset,
                    ap=[iap.ap[0], [0, NKT], iap.ap[1]],
                )
                nc.vector.tensor_tensor(
                    out=expT[:].rearrange("p (n d) -> p n d", d=QB),
                    in0=expT[:].rearrange("p (n d) -> p n d", d=QB),
                    in1=bc_ap, op=OP.mult,
                )
                for j in range(4):
                    nc.tensor.matmul(
                        accs[j][:],
                        ident[:],
                        expT[:, 512 * j:512 * (j + 1)],
                        start=(h == 0), stop=(h == H - 1),
                    )

            for h in range(H):
                head_tail(h, head_front(h))
            # attn_weights: evict acc (mean over heads), transpose to natural
            accq = accq_pool.tile([P, NKT * QB], F32R, tag="accq")
            for j in range(4):
                nc.scalar.mul(accq[:, 512 * j:512 * (j + 1)], accs[j][:], 1.0 / H)
            for kt in range(NKT):
                for qs in range(2):
                    tp = psum_av.tile([P, P], F32R, tag="av")
                    nc.tensor.transpose(
                        tp[:], accq[:, QB * kt + P * qs: QB * kt + P * (qs + 1)], ident[:]
                    )
                    wb = wnat_pool.tile([P, P], F32, tag="wnat", name=f"wb_{qb}_{kt}_{qs}")
                    nc.vector.tensor_copy(out=wb[:], in_=tp[:])
                    nc.sync.dma_start(
                        out=w_out[q0 + P * qs: q0 + P * (qs + 1), P * kt:P * (kt + 1)],
                        in_=wb[:],
                    )
            # out_proj + residual + LN
            x_qb = xqb_pool.tile([P, 2 * E], F32R, tag="xqb")
            for qs in range(2):
                nc.sync.dma_start(
                    out=x_qb[:, E * qs:E * (qs + 1)],
                    in_=x_q[q0 + P * qs: q0 + P * (qs + 1), :],
                )
            y_sb = ysb_pool.tile([P, 2 * E], F32, tag="ysb")
            for qs in range(2):
                for eb in range(2):
                    po = psum_acc.tile([P, 512], F32, tag="acc")
                    for c in range(NEC):
                        nc.tensor.matmul(
                            po[:],
                            attnT[:, QB * c + P * qs: QB * c + P * (qs + 1)],
                            woT[:, E * c + 512 * eb: E * c + 512 * (eb + 1)],
                            start=(c == 0), stop=False,
                        )
                    nc.tensor.matmul(
                        po[:], ones1[0:1, :],
                        bvbo_row[0:1, E + 512 * eb: E + 512 * (eb + 1)],
                        start=False, stop=True,
                    )
                    nc.vector.tensor_tensor(
                        out=y_sb[:, E * qs + 512 * eb: E * qs + 512 * (eb + 1)],
                        in0=po[:], in1=x_qb[:, E * qs + 512 * eb: E * qs + 512 * (eb + 1)],
                        op=OP.add,
                    )
                ych = y_sb[:, E * qs:E * (qs + 1)]
                stats = small.tile([P, 2, 6], F32, tag="stats")
                ychg = ych.rearrange("p (s f) -> p s f", f=512)
                for sg in range(2):
                    nc.vector.bn_stats(out=stats[:, sg, :], in_=ychg[:, sg, :])
                mv = small.tile([P, 2], F32, tag="mv")
                nc.vector.bn_aggr(out=mv[:], in_=stats[:])
                std = small.tile([P, 1], F32, tag="std")
                nc.scalar.activation(std[:], mv[:, 1:2], AF.Sqrt, bias=eps_sb[:])
                rstd = small.tile([P, 1], F32, tag="rstd")
                nc.vector.reciprocal(rstd[:], std[:])
                nc.vector.tensor_scalar(
                    out=ych, in0=ych, scalar1=mv[:, 0:1], scalar2=rstd[:],
                    op0=OP.subtract, op1=OP.mult,
                )
                nc.vector.tensor_tensor(out=ych, in0=ych, in1=gamma_bc[:], op=OP.mult)
                nc.vector.tensor_tensor(out=ych, in0=ych, in1=beta_bc[:], op=OP.add)
                nc.sync.dma_start(
                    out=y_out[q0 + P * qs: q0 + P * (qs + 1), :], in_=ych
                )


_CACHED = None


def _build():
    global _CACHED
    if _CACHED is not None:
        return _CACHED
    nc = bacc.Bacc("TRN2", target_bir_lowering=False, debug=False, num_devices=8)
    io = {}
    for name in ["x_q", "k_in", "v_in", "wqT", "wkT", "wvT", "woT"]:
        io[name] = nc.dram_tensor(name, [1024, 1024], F32R, kind="ExternalInput").ap()
    io["b_all"] = nc.dram_tensor("b_all", [4, 1024], F32R, kind="ExternalInput").ap()
    io["gb"] = nc.dram_tensor("gb", [2, 1024], F32R, kind="ExternalInput").ap()
    io["y_out"] = nc.dram_tensor("y_out", [1024, 1024], F32, kind="ExternalOutput").ap()
    io["w_out"] = nc.dram_tensor("w_out", [1024, 1024], F32, kind="ExternalOutput").ap()
    with tile.TileContext(nc) as tc:
        with ExitStack() as ctx:
            tc.ctx = ctx
            _emit(nc, tc, io)
    nc.compile()
    _CACHED = nc
    return nc


def kernel(query, key_t, value, in_proj_w, in_proj_b, out_proj_w, out_proj_b,
           ln_gamma, ln_beta, _trace=False, _tmpdir=None):
    query = np.ascontiguousarray(np.asarray(query, dtype=np.float32))
    key_t = np.ascontiguousarray(np.asarray(key_t, dtype=np.float32))
    value = np.ascontiguousarray(np.asarray(value, dtype=np.float32))
    in_proj_w = np.asarray(in_proj_w, dtype=np.float32)
    wqT = np.ascontiguousarray(in_proj_w[0:E].T)
    wkT = np.ascontiguousarray(in_proj_w[E:2 * E].T)
    wvT = np.ascontiguousarray(in_proj_w[2 * E:3 * E].T)
    woT = np.ascontiguousarray(np.asarray(out_proj_w, dtype=np.float32).T)
    b = np.asarray(in_proj_b, dtype=np.float32)
    b_all = np.ascontiguousarray(
        np.stack([b[0:E], b[E:2 * E], b[2 * E:3 * E],
                  np.asarray(out_proj_b, dtype=np.float32)])
    )
    gb = np.ascontiguousarray(
        np.stack([np.asarray(ln_gamma, dtype=np.float32),
                  np.asarray(ln_beta, dtype=np.float32)])
    )
    nc = _build()
    in_maps = [
        dict(x_q=query[c], k_in=key_t[c], v_in=value[c],
             wqT=wqT, wkT=wkT, wvT=wvT, woT=woT, b_all=b_all, gb=gb)
        for c in range(8)
    ]
    res = run_bass_kernel_spmd(
        nc, in_maps, core_ids=list(range(8)), trace=_trace, tmpdir=_tmpdir
    )
    y = np.stack([r["y_out"] for r in res.results])
    w = np.stack([r["w_out"] for r in res.results])
    kernel._last_result = res
    return y, w



# revision 45
# speedup vs baseline: 1.6570x; 1.6570x over previous
"""MultiHeadCrossAttention kernel for 8 Trainium2 NeuronCores.

Sharding: pure data-parallel over batch (B=8 -> 1 batch element per core).

v2 design (vs. 522us baseline):
  - Activations pre-transposed and downcast on the HOST (bf16 for the q/k
    scores path, fp8-e4m3 for the v path) -> no on-chip PE transposes.
  - q/k projections + scores + attn@v in bf16 (1 cycle/row on PE);
    v projection and out_proj in fp8 DoubleRow (0.5 cycles/row).
    The v/out path is scaled x16 host-side so fp8 operands sit in the
    normal range; compensated by a 1/256 scalar in the residual add.
  - exp on the Act engine only (no table swaps in the main loop), reading
    two PSUM banks per instruction, writing bf16 to SBUF.
  - softmax denominator via an extra ones-column in the v operand (row 64
    of the attn@v PSUM); reciprocal on DVE; broadcast on Pool.
  - attention-weights head-mean accumulated on DVE in bf16 (4 elem/cycle
    mode), not via PE identity matmuls; transposed back to natural [q,k]
    with DMA-XBAR transposes (14ns/tile, off all compute engines).
  - LayerNorm deferred to an epilogue so Sqrt causes one act-table load.
  - attn_weights returned as sum over heads; host divides by H=16.
"""

import numpy as np
import ml_dtypes
from contextlib import ExitStack

import concourse.bacc as bacc
import concourse.bass as bass
import concourse.tile as tile
from concourse import mybir
from concourse.bass_utils import run_bass_kernel_spmd
from concourse.masks import make_identity

E = 1024
H = 16
DH = 64
L = 1024
P = 128
QB = 256          # q-block size
NQB = L // QB     # 4
NKT = L // P      # 8 k-tiles
NEC = E // P      # 8 feature chunks
VS = H * (DH + 1)  # 1040 v columns per k-tile (65 per head)
LN_EPS = 1e-5

F32 = mybir.dt.float32
FP16 = mybir.dt.float16
E4M3 = mybir.dt.float8e4
AF = mybir.ActivationFunctionType
OP = mybir.AluOpType
DR = mybir.MatmulPerfMode.DoubleRow

NP_FP16 = np.float16
NP_E4M3 = ml_dtypes.float8_e4m3

# normalize runs on Pool for heads with h % 3 != 0 (DVE/Pool balance)


def _emit(nc, tc, io):
    ctx = tc.ctx
    ctx.enter_context(nc.allow_low_precision("fp16/fp8 attention"))

    const = ctx.enter_context(tc.tile_pool(name="const", bufs=1))
    persist = ctx.enter_context(tc.tile_pool(name="persist", bufs=1))

    ones1 = const.tile([1, P], FP16)
    nc.vector.memset(ones1[:], 1.0)
    one_u = const.tile([P, 1], mybir.dt.uint32)
    nc.vector.memset(one_u[:], 1)
    magic_u = const.tile([P, 1], mybir.dt.uint32)
    nc.vector.memset(magic_u[:], 0x5F3759DF)
    ident_f = const.tile([P, P], F32)
    make_identity(nc, ident_f[:])
    ident = const.tile([P, P], FP16)
    nc.vector.tensor_copy(ident[:], ident_f[:])

    # persistent activations / weights
    qT = persist.tile([P, NEC, L], FP16)     # [e%128, e//128, l]
    kT = persist.tile([P, NEC, L], FP16)
    v_sb = persist.tile([P, NKT * VS], FP16)  # [l%128, kt*(16 heads x 65)]
    wo8 = persist.tile([P, NEC, E], E4M3)    # 16*Wo.T  [e_in, e_out]

    # ones columns (softmax denominator trick)
    nc.vector.memset(
        v_sb[:].rearrange("p (n d) -> p n d", d=DH + 1)[:, :, DH:DH + 1], 1.0
    )

    ld_pool = ctx.enter_context(tc.tile_pool(name="ld", bufs=1))
    psum_p1 = ctx.enter_context(tc.tile_pool(name="psum_p1", bufs=2, space="PSUM"))
    psum_sc = ctx.enter_context(tc.tile_pool(name="psum_sc", bufs=2, space="PSUM"))
    psum_av = ctx.enter_context(tc.tile_pool(name="psum_av", bufs=2, space="PSUM"))
    expT_pool = ctx.enter_context(tc.tile_pool(name="expT", bufs=5))
    accq_pool = ctx.enter_context(tc.tile_pool(name="accq", bufs=4))
    a8_pool = ctx.enter_context(tc.tile_pool(name="a8", bufs=2))
    invbc_pool = ctx.enter_context(tc.tile_pool(name="invbc", bufs=6))
    wnat_pool = ctx.enter_context(tc.tile_pool(name="wnat", bufs=2))
    xqb_pool = ctx.enter_context(tc.tile_pool(name="xqb", bufs=2))
    small = ctx.enter_context(tc.tile_pool(name="small", bufs=4))
    z_pool = ctx.enter_context(tc.tile_pool(name="z16", bufs=2))
    ysb_pool = ctx.enter_context(tc.tile_pool(name="ysb", bufs=3))

    # ---- input loads, in dependency-criticality order ----
    vw = ld_pool.tile([P, 2 * NEC, E], E4M3, tag="aTx")
    aT_q = ld_pool.tile([P, NEC, L], FP16, tag="aTq")
    wt_q = ld_pool.tile([P, NEC, E], FP16, tag="wtq")
    nc.sync.dma_start(out=vw[:], in_=io["vw8"].rearrange("(c p) n -> p c n", p=P))
    # consts: one early DMA [1,4096] = [bvo(2048) | gamma | beta]; bqk cols
    crow = const.tile([1, 4 * E], FP16)
    nc.sync.dma_start(out=crow[:], in_=io["consts"][:])
    bvo_row = crow[:, 0:2 * E]
    g_row = crow[:, 2 * E:3 * E]
    b_row = crow[:, 3 * E:4 * E]
    bqk_col = const.tile([P, 2 * NEC], F32)

    for i in range(2):
        nc.sync.dma_start(
            out=bqk_col[:, NEC * i:NEC * (i + 1)],
            in_=io["bqk"][i, :].rearrange("(m p) -> p m", p=P),
        )
    nc.sync.dma_start(out=wt_q[:], in_=io["wq"].rearrange("(c p) n -> p c n", p=P))
    nc.sync.dma_start(out=aT_q[:], in_=io["xT"].rearrange("(c p) l -> p c l", p=P))
    gamma_bc = const.tile([P, E], FP16)
    beta_bc = const.tile([P, E], FP16)


    # ---- v projection (fp8 DoubleRow, x16 scale) ----
    for m in range(NEC):
        for n in range(2):
            ps = psum_p1.tile([P, 512], F32, tag="p1", name=f"pv_{m}_{n}")
            for sub in range(2):
                for pr in range(4):
                    nc.tensor.matmul(
                        ps[:, 256 * sub:256 * (sub + 1)],
                        vw[:, 8 + 2 * pr:8 + 2 * pr + 2, P * m:P * (m + 1)],
                        vw[:, 2 * pr:2 * pr + 2,
                           512 * n + 256 * sub:512 * n + 256 * (sub + 1)],
                        start=(pr == 0), stop=False,
                        perf_mode=DR,
                    )
                nc.tensor.matmul(
                    ps[:, 256 * sub:256 * (sub + 1)],
                    ones1[0:1, :],
                    bvo_row[:, 512 * n + 256 * sub:512 * n + 256 * (sub + 1)],
                    start=False, stop=True,
                )
            dst = v_sb[:, VS * m + 520 * n:VS * m + 520 * (n + 1)]
            nc.scalar.copy(
                out=dst.rearrange("p (h d) -> p h d", d=DH + 1)[:, :, 0:DH],
                in_=ps[:].rearrange("p (h d) -> p h d", d=DH),
            )

    nc.gpsimd.partition_broadcast(gamma_bc[:], g_row)
    nc.gpsimd.partition_broadcast(beta_bc[:], b_row)

    # k loads reuse the v buffers (freed by the v projection above)
    aT_k = ld_pool.tile([P, NEC, L], FP16, tag="aTx")
    wt_k = ld_pool.tile([P, NEC, E], FP16, tag="wtx")
    nc.sync.dma_start(out=wt_k[:], in_=io["wk"].rearrange("(c p) n -> p c n", p=P))
    nc.sync.dma_start(out=aT_k[:], in_=io["kTa"].rearrange("(c p) l -> p c l", p=P))
    nc.sync.dma_start(out=wo8[:], in_=io["wo8"].rearrange("(c p) n -> p c n", p=P))

    def qk_proj(ti, m):
        aT, wt = (aT_q, wt_q) if ti == 0 else (aT_k, wt_k)
        for n in range(2):
            ps = psum_p1.tile([P, 512], F32, tag="p1", name=f"p1_{ti}_{m}_{n}")
            for c in range(NEC):
                nc.tensor.matmul(
                    ps[:],
                    wt[:, c, P * m:P * (m + 1)],
                    aT[:, c, 512 * n:512 * (n + 1)],
                    start=(c == 0), stop=(c == NEC - 1),
                )
            dst = (qT if ti == 0 else kT)[:, m, 512 * n:512 * (n + 1)]
            nc.scalar.activation(
                dst, ps[:], AF.Identity,
                bias=bqk_col[:, NEC * ti + m:NEC * ti + m + 1],
            )

    # ---- per-qb state ----
    st = {}

    def qb_begin(qb):
        q0 = QB * qb
        x_qb = xqb_pool.tile([P, 2, E], FP16, tag="xqb", name=f"xqb_{qb}")
        nc.sync.dma_start(
            out=x_qb[:],
            in_=io["xnat"][q0:q0 + QB, :].rearrange("(s p) e -> p s e", p=P),
        )
        st[qb] = dict(
            x_qb=x_qb,
            Wacc=[accq_pool.tile([P, NKT * QB], FP16, tag="accq", name=f"wa_{qb}_{p}")
                  for p in range(2)],
            attnT8=a8_pool.tile([P, NEC, QB], E4M3, tag="attnT8", name=f"a8_{qb}"),
            ysb=ysb_pool.tile([P, 2, E], FP16, tag="ysb", name=f"y_{qb}"),
        )

    def head_front(qb, h):
        if h == 0:
            qb_begin(qb)
        q0 = QB * qb
        hb = (h % 2) * DH
        hc = h // 2
        expT = expT_pool.tile([P, NKT * QB], FP16, tag="expT",
                              name=f"expT_{qb}_{h}")
        for half in range(2):
            sc = psum_sc.tile([P, 1024], F32, tag="sc", name=f"sc_{qb}_{h}_{half}")
            for j in range(4):
                kt = 4 * half + j
                nc.tensor.matmul(
                    sc[:, QB * j:QB * (j + 1)],
                    kT[hb:hb + DH, hc, P * kt:P * (kt + 1)],
                    qT[hb:hb + DH, hc, q0:q0 + QB],
                    start=True, stop=True,
                )
            nc.scalar.activation(
                expT[:, 1024 * half:1024 * (half + 1)], sc[:],
                AF.Exp, scale=0.125,
            )
        return expT

    def tail_a(qb, h, expT):
        s = st[qb]
        hb = (h % 2) * DH
        hc = h // 2
        av = psum_av.tile([P, 512], F32, tag="av", name=f"av_{qb}_{h}")
        for kt in range(NKT):
            nc.tensor.matmul(
                av[0:DH + 1, 0:QB],
                v_sb[:, VS * kt + (DH + 1) * h:VS * kt + (DH + 1) * (h + 1)],
                expT[:, QB * kt:QB * (kt + 1)],
                start=(kt == 0), stop=(kt == NKT - 1),
            )
        inv = small.tile([1, QB], FP16, tag="inv", name=f"inv_{qb}_{h}")
        nc.vector.reciprocal(inv[:], av[DH:DH + 1, 0:QB])
        inv_bc = invbc_pool.tile([P, QB], FP16, tag="invbc", name=f"ib_{qb}_{h}")
        nc.gpsimd.partition_broadcast(inv_bc[:], inv[:])
        nc.vector.tensor_tensor(
            out=s["attnT8"][hb:hb + DH, hc, :],
            in0=av[0:DH, 0:QB], in1=inv_bc[0:DH, :], op=OP.mult,
        )
        return inv_bc

    def tail_b(qb, h, expT, inv_bc):
        s = st[qb]
        iap = inv_bc[:]
        bc_ap = bass.AP(tensor=iap.tensor, offset=iap.offset,
                        ap=[iap.ap[0], [0, NKT], iap.ap[1]])
        Wacc = s["Wacc"][h % 2]
        if h <= 1:
            nc.vector.tensor_tensor(
                out=Wacc[:].rearrange("p (n d) -> p n d", d=QB),
                in0=expT[:].rearrange("p (n d) -> p n d", d=QB),
                in1=bc_ap, op=OP.mult,
            )
        else:
            if h % 2 == 1:
                nc.gpsimd.tensor_tensor(
                    out=expT[:].rearrange("p (n d) -> p n d", d=QB),
                    in0=expT[:].rearrange("p (n d) -> p n d", d=QB),
                    in1=bc_ap, op=OP.mult,
                )
            else:
                nc.vector.tensor_tensor(
                    out=expT[:].rearrange("p (n d) -> p n d", d=QB),
                    in0=expT[:].rearrange("p (n d) -> p n d", d=QB),
                    in1=bc_ap, op=OP.mult,
                )
            nc.vector.tensor_tensor(out=Wacc[:], in0=Wacc[:], in1=expT[:],
                                     op=OP.add)

    def finalize_op(qb):
        s = st[qb]
        x_qb = s["x_qb"]
        attnT8 = s["attnT8"]
        for qs in range(2):
            for eb in range(2):
                po = psum_p1.tile([P, 512], F32, tag="p1", name=f"po_{qb}_{qs}_{eb}")
                for sub in range(2):
                    for pr in range(4):
                        nc.tensor.matmul(
                            po[:, 256 * sub:256 * (sub + 1)],
                            attnT8[:, 2 * pr:2 * pr + 2, P * qs:P * (qs + 1)],
                            wo8[:, 2 * pr:2 * pr + 2,
                                512 * eb + 256 * sub:512 * eb + 256 * (sub + 1)],
                            start=(pr == 0), stop=False,
                            perf_mode=DR,
                        )
                    nc.tensor.matmul(
                        po[:, 256 * sub:256 * (sub + 1)],
                        ones1[0:1, :],
                        bvo_row[:, E + 512 * eb + 256 * sub:
                                E + 512 * eb + 256 * (sub + 1)],
                        start=False, stop=True,
                    )
                nc.vector.scalar_tensor_tensor(
                    out=s["ysb"][:, qs, 512 * eb:512 * (eb + 1)],
                    in0=po[:], scalar=1.0 / 256.0,
                    in1=x_qb[:, qs, 512 * eb:512 * (eb + 1)],
                    op0=OP.mult, op1=OP.add,
                )

    def finalize_w(qb):
        s = st[qb]
        W0, W1 = s["Wacc"]
        nc.vector.tensor_tensor(out=W0[:], in0=W0[:], in1=W1[:], op=OP.add)
        Wacc = W0
        q0 = QB * qb
        # attn weights -> natural [q, k]; last qb transposes on the idle PE
        for qs in range(2):
            wnat = wnat_pool.tile([P, NKT, P], FP16, tag="wnat",
                                  name=f"wn_{qb}_{qs}")
            if qb == NQB - 1:
                tp = psum_p1.tile([P, NKT, P], FP16, tag="p1", name=f"tp_{qb}_{qs}")
                for kt in range(NKT):
                    nc.tensor.transpose(
                        tp[:, kt, :],
                        Wacc[:, QB * kt + P * qs:QB * kt + P * (qs + 1)],
                        ident[:],
                    )
                nc.vector.tensor_copy(wnat[:], tp[:])
            else:
                for kt in range(NKT):
                    nc.sync.dma_start_transpose(
                        wnat[:, kt, :],
                        Wacc[:, QB * kt + P * qs:QB * kt + P * (qs + 1)],
                    )
            nc.sync.dma_start(
                out=io["w16"][q0 + P * qs:q0 + P * (qs + 1), :], in_=wnat[:]
            )

    def finalize_ln(qb):
        # LayerNorm: batched stats + one-shot rsqrt (bit trick + 1 Newton)
        yqb = st[qb]["ysb"]
        mvs = []
        for qs in range(2):
            t = 2 * qb + qs
            stats = small.tile([P, 2, 6], F32, tag="stats", name=f"st_{t}")
            ychg = yqb[:, qs, :].rearrange("p (s f) -> p s f", f=512)
            for sg in range(2):
                nc.vector.bn_stats(out=stats[:, sg, :], in_=ychg[:, sg, :])
            mv = small.tile([P, 2], F32, tag="mv", name=f"mv_{t}")
            nc.vector.bn_aggr(out=mv[:], in_=stats[:])
            mvs.append(mv)
        ve = small.tile([P, 2], F32, tag="ve", name=f"ve_{qb}")
        for qs in range(2):
            nc.vector.tensor_scalar_add(out=ve[:, qs:qs + 1],
                                        in0=mvs[qs][:, 1:2], scalar1=LN_EPS)
        y0u = small.tile([P, 2], mybir.dt.uint32, tag="y0u", name=f"y0_{qb}")
        ou = bass.AP(tensor=one_u.tensor, offset=one_u[:].offset,
                     ap=[one_u[:].ap[0], [0, 2]])
        mu = bass.AP(tensor=magic_u.tensor, offset=magic_u[:].offset,
                     ap=[magic_u[:].ap[0], [0, 2]])
        nc.vector.tensor_tensor(out=y0u[:], in0=ve[:].bitcast(mybir.dt.uint32),
                                in1=ou, op=OP.logical_shift_right)
        nc.vector.tensor_tensor(out=y0u[:], in0=mu, in1=y0u[:], op=OP.subtract)
        y0 = y0u[:].bitcast(F32)
        rstd = small.tile([P, 2], F32, tag="rstd", name=f"rs_{qb}")
        tmp = small.tile([P, 2], F32, tag="tmp", name=f"tm_{qb}")
        nc.vector.tensor_tensor(out=tmp[:], in0=y0, in1=y0, op=OP.mult)
        nc.vector.tensor_tensor(out=tmp[:], in0=tmp[:], in1=ve[:], op=OP.mult)
        nc.vector.tensor_scalar(out=tmp[:], in0=tmp[:], scalar1=-0.5,
                                scalar2=1.5, op0=OP.mult, op1=OP.add)
        nc.vector.tensor_tensor(out=rstd[:], in0=y0, in1=tmp[:], op=OP.mult)
        for qs in range(2):
            t = 2 * qb + qs
            negmr = small.tile([P, 1], F32, tag="negmr", name=f"nm_{t}")
            nc.vector.tensor_tensor(out=negmr[:], in0=mvs[qs][:, 0:1],
                                    in1=rstd[:, qs:qs + 1], op=OP.mult)
            nc.vector.tensor_scalar_mul(out=negmr[:], in0=negmr[:], scalar1=-1.0)
            z16 = z_pool.tile([P, E], FP16, tag="z16", name=f"z_{t}")
            nc.scalar.activation(z16[:], yqb[:, qs, :], AF.Identity,
                                 bias=negmr[:], scale=rstd[:, qs:qs + 1])
            nc.vector.tensor_tensor(out=z16[:], in0=z16[:], in1=gamma_bc[:],
                                    op=OP.mult)
            nc.vector.tensor_tensor(out=z16[:], in0=z16[:], in1=beta_bc[:],
                                    op=OP.add)
            nc.sync.dma_start(out=io["y16"][P * t:P * (t + 1), :], in_=z16[:])

    # ---- fused pipeline: q-proj, then k-proj interleaved with heads ----
    for m in range(3):
        qk_proj(0, m)

    jobs = [(qb, h) for qb in range(NQB) for h in range(H)]
    N = len(jobs)
    fronts = {}
    invs = {}
    nf = 0
    na = 0
    nb = 0

    def emit_front():
        nonlocal nf
        qb, h = jobs[nf]
        fronts[nf] = head_front(qb, h)
        nf += 1

    def emit_a():
        nonlocal na
        qb, h = jobs[na]
        invs[na] = tail_a(qb, h, fronts[na])
        na += 1
        if h == H - 1:
            finalize_op(qb)
            if qb == NQB - 1:
                finalize_ln(qb)

    def emit_b():
        nonlocal nb
        qb, h = jobs[nb]
        tail_b(qb, h, fronts.pop(nb), invs.pop(nb))
        nb += 1
        if h == H - 1:
            finalize_w(qb)
        if nb >= 6 and (nb - 6) % H == 0 and 1 <= (nb - 6) // H < NQB:
            finalize_ln((nb - 6) // H - 1)

    def step():
        if nf < N:
            emit_front()
        if na < min(nf - 1, N) if nf < N else na < N:
            emit_a()
        if nb < min(na - 1, N) if na < N else nb < N:
            emit_b()

    for m in range(NEC):
        qk_proj(1, m)
        emit_front()
        emit_front()
        if m >= 1:
            emit_a()
            emit_a()
        if m >= 2:
            emit_b()
            emit_b()
        if m + 3 < NEC:
            qk_proj(0, m + 3)
    while nb < N:
        if nf < N:
            emit_front()
        if na < nf - 1 or (nf == N and na < N):
            emit_a()
        if nb < na - 1 or (na == N and nb < N):
            emit_b()


# revision 46
# speedup vs baseline: 1.6742x; 1.0104x over previous
"""MultiHeadCrossAttention kernel for 8 Trainium2 NeuronCores.

Sharding: pure data-parallel over batch (B=8 -> 1 batch element per core).

Final design, 314,992ns vs 522,000ns baseline (1.66x), HW-verified:
  - Activations pre-transposed and downcast on the HOST (fp16 for the q/k
    scores path, fp8-e4m3 for the v path) -> no on-chip PE transposes.
  - q/k projections + scores + attn@v in fp16 (1 cycle/row on PE);
    v projection and out_proj in fp8 DoubleRow (0.5 cycles/row), scaled
    x16 host-side so fp8 operands are normal; 1/256 in the residual STT.
  - Single fused software pipeline: v-proj, then q/k projections
    interleaved with the 64 (q-block, head) attention jobs in three
    stages (scores/exp -> attn.v/recip/broadcast -> normalize/accumulate)
    so latency-critical small ops never queue behind bulk ops.
  - exp on Act only (one act-table load total), two PSUM banks per read.
  - softmax denominator via a ones-column in the v operand; reciprocal
    on DVE; partition-broadcast on Pool (Pool cannot read PSUM on HW).
  - attn-weights head-mean: two parity accumulation chains (even on DVE,
    odd heads normalized on Pool) merged per q-block; natural [q,k]
    layout via DMA-XBAR transposes (PE transposes for the last q-block).
  - LayerNorm inline per q-block, deferred 6 heads to protect the exp
    stream; rsqrt via uint32 bit trick + Newton on DVE (no Sqrt table).
  - attn_weights returned as sum over heads; host divides by H=16.
"""

import numpy as np
import ml_dtypes
from contextlib import ExitStack

import concourse.bacc as bacc
import concourse.bass as bass
import concourse.tile as tile
from concourse import mybir
from concourse.bass_utils import run_bass_kernel_spmd
from concourse.masks import make_identity

E = 1024
H = 16
DH = 64
L = 1024
P = 128
QB = 256          # q-block size
NQB = L // QB     # 4
NKT = L // P      # 8 k-tiles
NEC = E // P      # 8 feature chunks
VS = H * (DH + 1)  # 1040 v columns per k-tile (65 per head)
LN_EPS = 1e-5

F32 = mybir.dt.float32
FP16 = mybir.dt.float16
E4M3 = mybir.dt.float8e4
AF = mybir.ActivationFunctionType
OP = mybir.AluOpType
DR = mybir.MatmulPerfMode.DoubleRow

NP_FP16 = np.float16
NP_E4M3 = ml_dtypes.float8_e4m3

# normalize runs on Pool for heads with h % 3 != 0 (DVE/Pool balance)


def _emit(nc, tc, io):
    ctx = tc.ctx
    ctx.enter_context(nc.allow_low_precision("fp16/fp8 attention"))

    const = ctx.enter_context(tc.tile_pool(name="const", bufs=1))
    persist = ctx.enter_context(tc.tile_pool(name="persist", bufs=1))

    ones1 = const.tile([1, P], FP16)
    nc.vector.memset(ones1[:], 1.0)
    one_u = const.tile([P, 1], mybir.dt.uint32)
    nc.vector.memset(one_u[:], 1)
    magic_u = const.tile([P, 1], mybir.dt.uint32)
    nc.vector.memset(magic_u[:], 0x5F3759DF)
    ident_f = const.tile([P, P], F32)
    make_identity(nc, ident_f[:])
    ident = const.tile([P, P], FP16)
    nc.vector.tensor_copy(ident[:], ident_f[:])

    # persistent activations / weights
    qT = persist.tile([P, NEC, L], FP16)     # [e%128, e//128, l]
    kT = persist.tile([P, NEC, L], FP16)
    v_sb = persist.tile([P, NKT * VS], FP16)  # [l%128, kt*(16 heads x 65)]
    wo8 = persist.tile([P, NEC, E], E4M3)    # 16*Wo.T  [e_in, e_out]

    # ones columns (softmax denominator trick)
    nc.vector.memset(
        v_sb[:].rearrange("p (n d) -> p n d", d=DH + 1)[:, :, DH:DH + 1], 1.0
    )

    ld_pool = ctx.enter_context(tc.tile_pool(name="ld", bufs=1))
    psum_p1 = ctx.enter_context(tc.tile_pool(name="psum_p1", bufs=2, space="PSUM"))
    psum_sc = ctx.enter_context(tc.tile_pool(name="psum_sc", bufs=2, space="PSUM"))
    psum_av = ctx.enter_context(tc.tile_pool(name="psum_av", bufs=2, space="PSUM"))
    expT_pool = ctx.enter_context(tc.tile_pool(name="expT", bufs=5))
    accq_pool = ctx.enter_context(tc.tile_pool(name="accq", bufs=4))
    a8_pool = ctx.enter_context(tc.tile_pool(name="a8", bufs=2))
    invbc_pool = ctx.enter_context(tc.tile_pool(name="invbc", bufs=6))
    wnat_pool = ctx.enter_context(tc.tile_pool(name="wnat", bufs=2))
    xqb_pool = ctx.enter_context(tc.tile_pool(name="xqb", bufs=2))
    small = ctx.enter_context(tc.tile_pool(name="small", bufs=4))
    z_pool = ctx.enter_context(tc.tile_pool(name="z16", bufs=2))
    ysb_pool = ctx.enter_context(tc.tile_pool(name="ysb", bufs=3))

    # ---- input loads, in dependency-criticality order ----
    vw = ld_pool.tile([P, 2 * NEC, E], E4M3, tag="aTx")
    aT_q = ld_pool.tile([P, NEC, L], FP16, tag="aTq")
    wt_q = ld_pool.tile([P, NEC, E], FP16, tag="wtq")
    nc.sync.dma_start(out=vw[:], in_=io["vw8"].rearrange("(c p) n -> p c n", p=P))
    # consts: one early DMA [1,4096] = [bvo(2048) | gamma | beta]; bqk cols
    crow = const.tile([1, 4 * E], FP16)
    nc.sync.dma_start(out=crow[:], in_=io["consts"][:])
    bvo_row = crow[:, 0:2 * E]
    g_row = crow[:, 2 * E:3 * E]
    b_row = crow[:, 3 * E:4 * E]
    bqk_col = const.tile([P, 2 * NEC], F32)

    for i in range(2):
        nc.sync.dma_start(
            out=bqk_col[:, NEC * i:NEC * (i + 1)],
            in_=io["bqk"][i, :].rearrange("(m p) -> p m", p=P),
        )
    nc.sync.dma_start(out=wt_q[:], in_=io["wq"].rearrange("(c p) n -> p c n", p=P))
    nc.sync.dma_start(out=aT_q[:], in_=io["xT"].rearrange("(c p) l -> p c l", p=P))
    gamma_bc = const.tile([P, E], FP16)
    beta_bc = const.tile([P, E], FP16)


    # ---- v projection (fp8 DoubleRow, x16 scale) ----
    for m in range(NEC):
        for n in range(2):
            ps = psum_p1.tile([P, 512], F32, tag="p1", name=f"pv_{m}_{n}")
            for sub in range(2):
                for pr in range(4):
                    nc.tensor.matmul(
                        ps[:, 256 * sub:256 * (sub + 1)],
                        vw[:, 8 + 2 * pr:8 + 2 * pr + 2, P * m:P * (m + 1)],
                        vw[:, 2 * pr:2 * pr + 2,
                           512 * n + 256 * sub:512 * n + 256 * (sub + 1)],
                        start=(pr == 0), stop=False,
                        perf_mode=DR,
                    )
                nc.tensor.matmul(
                    ps[:, 256 * sub:256 * (sub + 1)],
                    ones1[0:1, :],
                    bvo_row[:, 512 * n + 256 * sub:512 * n + 256 * (sub + 1)],
                    start=False, stop=True,
                )
            dst = v_sb[:, VS * m + 520 * n:VS * m + 520 * (n + 1)]
            nc.scalar.copy(
                out=dst.rearrange("p (h d) -> p h d", d=DH + 1)[:, :, 0:DH],
                in_=ps[:].rearrange("p (h d) -> p h d", d=DH),
            )

    nc.gpsimd.partition_broadcast(gamma_bc[:], g_row)
    nc.gpsimd.partition_broadcast(beta_bc[:], b_row)

    # k loads reuse the v buffers (freed by the v projection above)
    aT_k = ld_pool.tile([P, NEC, L], FP16, tag="aTx")
    wt_k = ld_pool.tile([P, NEC, E], FP16, tag="wtx")
    nc.sync.dma_start(out=wt_k[:], in_=io["wk"].rearrange("(c p) n -> p c n", p=P))
    nc.sync.dma_start(out=aT_k[:], in_=io["kTa"].rearrange("(c p) l -> p c l", p=P))
    nc.sync.dma_start(out=wo8[:], in_=io["wo8"].rearrange("(c p) n -> p c n", p=P))

    def qk_proj(ti, m):
        aT, wt = (aT_q, wt_q) if ti == 0 else (aT_k, wt_k)
        for n in range(2):
            ps = psum_p1.tile([P, 512], F32, tag="p1", name=f"p1_{ti}_{m}_{n}")
            for c in range(NEC):
                nc.tensor.matmul(
                    ps[:],
                    wt[:, c, P * m:P * (m + 1)],
                    aT[:, c, 512 * n:512 * (n + 1)],
                    start=(c == 0), stop=(c == NEC - 1),
                )
            dst = (qT if ti == 0 else kT)[:, m, 512 * n:512 * (n + 1)]
            nc.scalar.activation(
                dst, ps[:], AF.Identity,
                bias=bqk_col[:, NEC * ti + m:NEC * ti + m + 1],
            )

    # ---- per-qb state ----
    st = {}

    def qb_begin(qb):
        q0 = QB * qb
        x_qb = xqb_pool.tile([P, 2, E], FP16, tag="xqb", name=f"xqb_{qb}")
        nc.sync.dma_start(
            out=x_qb[:],
            in_=io["xnat"][q0:q0 + QB, :].rearrange("(s p) e -> p s e", p=P),
        )
        st[qb] = dict(
            x_qb=x_qb,
            Wacc=[accq_pool.tile([P, NKT * QB], FP16, tag="accq", name=f"wa_{qb}_{p}")
                  for p in range(2)],
            attnT8=a8_pool.tile([P, NEC, QB], E4M3, tag="attnT8", name=f"a8_{qb}"),
            ysb=ysb_pool.tile([P, 2, E], FP16, tag="ysb", name=f"y_{qb}"),
        )

    def head_front(qb, h):
        if h == 0:
            qb_begin(qb)
        q0 = QB * qb
        hb = (h % 2) * DH
        hc = h // 2
        expT = expT_pool.tile([P, NKT * QB], FP16, tag="expT",
                              name=f"expT_{qb}_{h}")
        for half in range(2):
            sc = psum_sc.tile([P, 1024], F32, tag="sc", name=f"sc_{qb}_{h}_{half}")
            for j in range(4):
                kt = 4 * half + j
                nc.tensor.matmul(
                    sc[:, QB * j:QB * (j + 1)],
                    kT[hb:hb + DH, hc, P * kt:P * (kt + 1)],
                    qT[hb:hb + DH, hc, q0:q0 + QB],
                    start=True, stop=True,
                )
            nc.scalar.activation(
                expT[:, 1024 * half:1024 * (half + 1)], sc[:],
                AF.Exp, scale=0.125,
            )
        return expT

    def tail_a(qb, h, expT):
        s = st[qb]
        hb = (h % 2) * DH
        hc = h // 2
        av = psum_av.tile([P, 512], F32, tag="av", name=f"av_{qb}_{h}")
        for kt in range(NKT):
            nc.tensor.matmul(
                av[0:DH + 1, 0:QB],
                v_sb[:, VS * kt + (DH + 1) * h:VS * kt + (DH + 1) * (h + 1)],
                expT[:, QB * kt:QB * (kt + 1)],
                start=(kt == 0), stop=(kt == NKT - 1),
            )
        inv = small.tile([1, QB], FP16, tag="inv", name=f"inv_{qb}_{h}")
        nc.vector.reciprocal(inv[:], av[DH:DH + 1, 0:QB])
        inv_bc = invbc_pool.tile([P, QB], FP16, tag="invbc", name=f"ib_{qb}_{h}")
        nc.gpsimd.partition_broadcast(inv_bc[:], inv[:])
        nc.vector.tensor_tensor(
            out=s["attnT8"][hb:hb + DH, hc, :],
            in0=av[0:DH, 0:QB], in1=inv_bc[0:DH, :], op=OP.mult,
        )
        return inv_bc

    def tail_b(qb, h, expT, inv_bc):
        s = st[qb]
        iap = inv_bc[:]
        bc_ap = bass.AP(tensor=iap.tensor, offset=iap.offset,
                        ap=[iap.ap[0], [0, NKT], iap.ap[1]])
        Wacc = s["Wacc"][h % 2]
        if h <= 1:
            nc.vector.tensor_tensor(
                out=Wacc[:].rearrange("p (n d) -> p n d", d=QB),
                in0=expT[:].rearrange("p (n d) -> p n d", d=QB),
                in1=bc_ap, op=OP.mult,
            )
        else:
            if h % 2 == 1:
                nc.gpsimd.tensor_tensor(
                    out=expT[:].rearrange("p (n d) -> p n d", d=QB),
                    in0=expT[:].rearrange("p (n d) -> p n d", d=QB),
                    in1=bc_ap, op=OP.mult,
                )
            else:
                nc.vector.tensor_tensor(
                    out=expT[:].rearrange("p (n d) -> p n d", d=QB),
                    in0=expT[:].rearrange("p (n d) -> p n d", d=QB),
                    in1=bc_ap, op=OP.mult,
                )
            nc.vector.tensor_tensor(out=Wacc[:], in0=Wacc[:], in1=expT[:],
                                     op=OP.add)

    def finalize_op(qb):
        s = st[qb]
        x_qb = s["x_qb"]
        attnT8 = s["attnT8"]
        for qs in range(2):
            for eb in range(2):
                po = psum_p1.tile([P, 512], F32, tag="p1", name=f"po_{qb}_{qs}_{eb}")
                for sub in range(2):
                    for pr in range(4):
                        nc.tensor.matmul(
                            po[:, 256 * sub:256 * (sub + 1)],
                            attnT8[:, 2 * pr:2 * pr + 2, P * qs:P * (qs + 1)],
                            wo8[:, 2 * pr:2 * pr + 2,
                                512 * eb + 256 * sub:512 * eb + 256 * (sub + 1)],
                            start=(pr == 0), stop=False,
                            perf_mode=DR,
                        )
                    nc.tensor.matmul(
                        po[:, 256 * sub:256 * (sub + 1)],
                        ones1[0:1, :],
                        bvo_row[:, E + 512 * eb + 256 * sub:
                                E + 512 * eb + 256 * (sub + 1)],
                        start=False, stop=True,
                    )
                nc.vector.scalar_tensor_tensor(
                    out=s["ysb"][:, qs, 512 * eb:512 * (eb + 1)],
                    in0=po[:], scalar=1.0 / 256.0,
                    in1=x_qb[:, qs, 512 * eb:512 * (eb + 1)],
                    op0=OP.mult, op1=OP.add,
                )

    def finalize_w(qb):
        s = st[qb]
        W0, W1 = s["Wacc"]
        nc.vector.tensor_tensor(out=W0[:], in0=W0[:], in1=W1[:], op=OP.add)
        Wacc = W0
        q0 = QB * qb
        # attn weights -> natural [q, k]; last qb transposes on the idle PE
        for qs in range(2):
            wnat = wnat_pool.tile([P, NKT, P], FP16, tag="wnat",
                                  name=f"wn_{qb}_{qs}")
            if qb == NQB - 1:
                tp = psum_p1.tile([P, NKT, P], FP16, tag="p1", name=f"tp_{qb}_{qs}")
                for kt in range(NKT):
                    nc.tensor.transpose(
                        tp[:, kt, :],
                        Wacc[:, QB * kt + P * qs:QB * kt + P * (qs + 1)],
                        ident[:],
                    )
                nc.vector.tensor_copy(wnat[:], tp[:])
            else:
                for kt in range(NKT):
                    nc.sync.dma_start_transpose(
                        wnat[:, kt, :],
                        Wacc[:, QB * kt + P * qs:QB * kt + P * (qs + 1)],
                    )
            nc.sync.dma_start(
                out=io["w16"][q0 + P * qs:q0 + P * (qs + 1), :], in_=wnat[:]
            )

    def finalize_ln(qb):
        # LayerNorm: batched stats + one-shot rsqrt (bit trick + 1 Newton)
        yqb = st[qb]["ysb"]
        mvs = []
        for qs in range(2):
            t = 2 * qb + qs
            stats = small.tile([P, 2, 6], F32, tag="stats", name=f"st_{t}")
            ychg = yqb[:, qs, :].rearrange("p (s f) -> p s f", f=512)
            for sg in range(2):
                nc.vector.bn_stats(out=stats[:, sg, :], in_=ychg[:, sg, :])
            mv = small.tile([P, 2], F32, tag="mv", name=f"mv_{t}")
            nc.vector.bn_aggr(out=mv[:], in_=stats[:])
            mvs.append(mv)
        ve = small.tile([P, 2], F32, tag="ve", name=f"ve_{qb}")
        for qs in range(2):
            nc.vector.tensor_scalar_add(out=ve[:, qs:qs + 1],
                                        in0=mvs[qs][:, 1:2], scalar1=LN_EPS)
        y0u = small.tile([P, 2], mybir.dt.uint32, tag="y0u", name=f"y0_{qb}")
        ou = bass.AP(tensor=one_u.tensor, offset=one_u[:].offset,
                     ap=[one_u[:].ap[0], [0, 2]])
        mu = bass.AP(tensor=magic_u.tensor, offset=magic_u[:].offset,
                     ap=[magic_u[:].ap[0], [0, 2]])
        nc.vector.tensor_tensor(out=y0u[:], in0=ve[:].bitcast(mybir.dt.uint32),
                                in1=ou, op=OP.logical_shift_right)
        nc.vector.tensor_tensor(out=y0u[:], in0=mu, in1=y0u[:], op=OP.subtract)
        y0 = y0u[:].bitcast(F32)
        rstd = small.tile([P, 2], F32, tag="rstd", name=f"rs_{qb}")
        tmp = small.tile([P, 2], F32, tag="tmp", name=f"tm_{qb}")
        nc.vector.tensor_tensor(out=tmp[:], in0=y0, in1=y0, op=OP.mult)
        nc.vector.tensor_tensor(out=tmp[:], in0=tmp[:], in1=ve[:], op=OP.mult)
        nc.vector.tensor_scalar(out=tmp[:], in0=tmp[:], scalar1=-0.5,
                                scalar2=1.5, op0=OP.mult, op1=OP.add)
        nc.vector.tensor_tensor(out=rstd[:], in0=y0, in1=tmp[:], op=OP.mult)
        for qs in range(2):
            t = 2 * qb + qs
            negmr = small.tile([P, 1], F32, tag="negmr", name=f"nm_{t}")
            nc.vector.tensor_tensor(out=negmr[:], in0=mvs[qs][:, 0:1],
                                    in1=rstd[:, qs:qs + 1], op=OP.mult)
            nc.vector.tensor_scalar_mul(out=negmr[:], in0=negmr[:], scalar1=-1.0)
            z16 = z_pool.tile([P, E], FP16, tag="z16", name=f"z_{t}")
            nc.scalar.activation(z16[:], yqb[:, qs, :], AF.Identity,
                                 bias=negmr[:], scale=rstd[:, qs:qs + 1])
            nc.vector.tensor_tensor(out=z16[:], in0=z16[:], in1=gamma_bc[:],
                                    op=OP.mult)
            nc.vector.tensor_tensor(out=z16[:], in0=z16[:], in1=beta_bc[:],
                                    op=OP.add)
            nc.sync.dma_start(out=io["y16"][P * t:P * (t + 1), :], in_=z16[:])

    # ---- fused pipeline: q-proj, then k-proj interleaved with heads ----
    for m in range(3):
        qk_proj(0, m)

    jobs = [(qb, h) for qb in range(NQB) for h in range(H)]
    N = len(jobs)
    fronts = {}
    invs = {}
    nf = 0
    na = 0
    nb = 0

    def emit_front():
        nonlocal nf
        qb, h = jobs[nf]
        fronts[nf] = head_front(qb, h)
        nf += 1

    def emit_a():
        nonlocal na
        qb, h = jobs[na]
        invs[na] = tail_a(qb, h, fronts[na])
        na += 1
        if h == H - 1:
            finalize_op(qb)
            if qb == NQB - 1:
                finalize_ln(qb)

    def emit_b():
        nonlocal nb
        qb, h = jobs[nb]
        tail_b(qb, h, fronts.pop(nb), invs.pop(nb))
        nb += 1
        if h == H - 1:
            finalize_w(qb)
        if nb >= 6 and (nb - 6) % H == 0 and 1 <= (nb - 6) // H < NQB:
            finalize_ln((nb - 6) // H - 1)

    def step():
        if nf < N:
            emit_front()
        if na < min(nf - 1, N) if nf < N else na < N:
            emit_a()
        if nb < min(na - 1, N) if na < N else nb < N:
            emit_b()

    for m in range(NEC):
        qk_proj(1, m)
        emit_front()
        emit_front()
        if m >= 1:
            emit_a()
            emit_a()
        if m >= 2:
            emit_b()
            emit_b()
        if m + 3 < NEC:
            qk_proj(0, m + 3)
    while nb < N:
        if nf < N:
            emit_front()
        if na < nf - 1 or (nf == N and na < N):
            emit_a()
        if nb < na - 1 or (na == N and nb < N):
            emit_b()


# revision 47
# speedup vs baseline: 1.6786x; 1.0026x over previous
"""MultiHeadCrossAttention kernel for 8 Trainium2 NeuronCores.

Sharding: pure data-parallel over batch (B=8 -> 1 batch element per core).

Final design, 314,992ns vs 522,000ns baseline (1.66x), HW-verified:
  - Activations pre-transposed and downcast on the HOST (fp16 for the q/k
    scores path, fp8-e4m3 for the v path) -> no on-chip PE transposes.
  - q/k projections + scores + attn@v in fp16 (1 cycle/row on PE);
    v projection and out_proj in fp8 DoubleRow (0.5 cycles/row), scaled
    x16 host-side so fp8 operands are normal; 1/256 in the residual STT.
  - Single fused software pipeline: v-proj, then q/k projections
    interleaved with the 64 (q-block, head) attention jobs in three
    stages (scores/exp -> attn.v/recip/broadcast -> normalize/accumulate)
    so latency-critical small ops never queue behind bulk ops.
  - exp on Act only (one act-table load total), two PSUM banks per read.
  - softmax denominator via a ones-column in the v operand; reciprocal
    on DVE; partition-broadcast on Pool (Pool cannot read PSUM on HW).
  - attn-weights head-mean: two parity accumulation chains (even on DVE,
    odd heads normalized on Pool) merged per q-block; natural [q,k]
    layout via DMA-XBAR transposes (PE transposes for the last q-block).
  - LayerNorm inline per q-block, deferred 6 heads to protect the exp
    stream; rsqrt via uint32 bit trick + Newton on DVE (no Sqrt table).
  - attn_weights returned as sum over heads; host divides by H=16.
"""

import numpy as np
import ml_dtypes
from contextlib import ExitStack

import concourse.bacc as bacc
import concourse.bass as bass
import concourse.tile as tile
from concourse import mybir
from concourse.bass_utils import run_bass_kernel_spmd
from concourse.masks import make_identity

E = 1024
H = 16
DH = 64
L = 1024
P = 128
QB = 256          # q-block size
NQB = L // QB     # 4
NKT = L // P      # 8 k-tiles
NEC = E // P      # 8 feature chunks
VS = H * (DH + 1)  # 1040 v columns per k-tile (65 per head)
LN_EPS = 1e-5

F32 = mybir.dt.float32
FP16 = mybir.dt.float16
E4M3 = mybir.dt.float8e4
AF = mybir.ActivationFunctionType
OP = mybir.AluOpType
DR = mybir.MatmulPerfMode.DoubleRow

NP_FP16 = np.float16
NP_E4M3 = ml_dtypes.float8_e4m3

# normalize runs on Pool for heads with h % 3 != 0 (DVE/Pool balance)


def _emit(nc, tc, io):
    ctx = tc.ctx
    ctx.enter_context(nc.allow_low_precision("fp16/fp8 attention"))

    const = ctx.enter_context(tc.tile_pool(name="const", bufs=1))
    persist = ctx.enter_context(tc.tile_pool(name="persist", bufs=1))

    ones1 = const.tile([1, P], FP16)
    nc.vector.memset(ones1[:], 1.0)
    one_u = const.tile([P, 1], mybir.dt.uint32)
    nc.vector.memset(one_u[:], 1)
    magic_u = const.tile([P, 1], mybir.dt.uint32)
    nc.vector.memset(magic_u[:], 0x5F3759DF)
    ident_f = const.tile([P, P], F32)
    make_identity(nc, ident_f[:])
    ident = const.tile([P, P], FP16)
    nc.vector.tensor_copy(ident[:], ident_f[:])

    # persistent activations / weights
    qT = persist.tile([P, NEC, L], FP16)     # [e%128, e//128, l]
    kT = persist.tile([P, NEC, L], FP16)
    v_sb = persist.tile([P, NKT * VS], FP16)  # [l%128, kt*(16 heads x 65)]
    wo8 = persist.tile([P, NEC, E], E4M3)    # 16*Wo.T  [e_in, e_out]

    # ones columns (softmax denominator trick)
    nc.vector.memset(
        v_sb[:].rearrange("p (n d) -> p n d", d=DH + 1)[:, :, DH:DH + 1], 1.0
    )

    ld_pool = ctx.enter_context(tc.tile_pool(name="ld", bufs=1))
    psum_p1 = ctx.enter_context(tc.tile_pool(name="psum_p1", bufs=2, space="PSUM"))
    psum_sc = ctx.enter_context(tc.tile_pool(name="psum_sc", bufs=2, space="PSUM"))
    psum_av = ctx.enter_context(tc.tile_pool(name="psum_av", bufs=2, space="PSUM"))
    expT_pool = ctx.enter_context(tc.tile_pool(name="expT", bufs=5))
    accq_pool = ctx.enter_context(tc.tile_pool(name="accq", bufs=4))
    a8_pool = ctx.enter_context(tc.tile_pool(name="a8", bufs=2))
    invbc_pool = ctx.enter_context(tc.tile_pool(name="invbc", bufs=6))
    wnat_pool = ctx.enter_context(tc.tile_pool(name="wnat", bufs=2))
    xqb_pool = ctx.enter_context(tc.tile_pool(name="xqb", bufs=2))
    small = ctx.enter_context(tc.tile_pool(name="small", bufs=4))
    z_pool = ctx.enter_context(tc.tile_pool(name="z16", bufs=2))
    ysb_pool = ctx.enter_context(tc.tile_pool(name="ysb", bufs=3))

    # ---- input loads, in dependency-criticality order ----
    vw = ld_pool.tile([P, 2 * NEC, E], E4M3, tag="aTx")
    aT_q = ld_pool.tile([P, NEC, L], FP16, tag="aTq")
    wt_q = ld_pool.tile([P, NEC, E], FP16, tag="wtq")
    nc.sync.dma_start(out=vw[:], in_=io["vw8"].rearrange("(c p) n -> p c n", p=P))
    # consts: one early DMA [1,4096] = [bvo(2048) | gamma | beta]; bqk cols
    crow = const.tile([1, 4 * E], FP16)
    nc.sync.dma_start(out=crow[:], in_=io["consts"][:])
    bvo_row = crow[:, 0:2 * E]
    g_row = crow[:, 2 * E:3 * E]
    b_row = crow[:, 3 * E:4 * E]
    bqk_col = const.tile([P, 2 * NEC], F32)

    for i in range(2):
        nc.sync.dma_start(
            out=bqk_col[:, NEC * i:NEC * (i + 1)],
            in_=io["bqk"][i, :].rearrange("(m p) -> p m", p=P),
        )
    nc.sync.dma_start(out=wt_q[:], in_=io["wq"].rearrange("(c p) n -> p c n", p=P))
    nc.sync.dma_start(out=aT_q[:], in_=io["xT"].rearrange("(c p) l -> p c l", p=P))
    gamma_bc = const.tile([P, E], FP16)
    beta_bc = const.tile([P, E], FP16)


    # ---- v projection (fp8 DoubleRow, x16 scale) ----
    for m in range(NEC):
        for n in range(2):
            ps = psum_p1.tile([P, 512], F32, tag="p1", name=f"pv_{m}_{n}")
            for sub in range(2):
                for pr in range(4):
                    nc.tensor.matmul(
                        ps[:, 256 * sub:256 * (sub + 1)],
                        vw[:, 8 + 2 * pr:8 + 2 * pr + 2, P * m:P * (m + 1)],
                        vw[:, 2 * pr:2 * pr + 2,
                           512 * n + 256 * sub:512 * n + 256 * (sub + 1)],
                        start=(pr == 0), stop=False,
                        perf_mode=DR,
                    )
                nc.tensor.matmul(
                    ps[:, 256 * sub:256 * (sub + 1)],
                    ones1[0:1, :],
                    bvo_row[:, 512 * n + 256 * sub:512 * n + 256 * (sub + 1)],
                    start=False, stop=True,
                )
            dst = v_sb[:, VS * m + 520 * n:VS * m + 520 * (n + 1)]
            nc.scalar.copy(
                out=dst.rearrange("p (h d) -> p h d", d=DH + 1)[:, :, 0:DH],
                in_=ps[:].rearrange("p (h d) -> p h d", d=DH),
            )

    nc.gpsimd.partition_broadcast(gamma_bc[:], g_row)
    nc.gpsimd.partition_broadcast(beta_bc[:], b_row)

    # k loads reuse the v buffers (freed by the v projection above)
    aT_k = ld_pool.tile([P, NEC, L], FP16, tag="aTx")
    wt_k = ld_pool.tile([P, NEC, E], FP16, tag="wtx")
    nc.sync.dma_start(out=wt_k[:], in_=io["wk"].rearrange("(c p) n -> p c n", p=P))
    nc.sync.dma_start(out=aT_k[:], in_=io["kTa"].rearrange("(c p) l -> p c l", p=P))
    nc.sync.dma_start(out=wo8[:], in_=io["wo8"].rearrange("(c p) n -> p c n", p=P))

    def qk_proj(ti, m):
        aT, wt = (aT_q, wt_q) if ti == 0 else (aT_k, wt_k)
        for n in range(2):
            ps = psum_p1.tile([P, 512], F32, tag="p1", name=f"p1_{ti}_{m}_{n}")
            for c in range(NEC):
                nc.tensor.matmul(
                    ps[:],
                    wt[:, c, P * m:P * (m + 1)],
                    aT[:, c, 512 * n:512 * (n + 1)],
                    start=(c == 0), stop=(c == NEC - 1),
                )
            dst = (qT if ti == 0 else kT)[:, m, 512 * n:512 * (n + 1)]
            nc.scalar.activation(
                dst, ps[:], AF.Identity,
                bias=bqk_col[:, NEC * ti + m:NEC * ti + m + 1],
            )

    # ---- per-qb state ----
    st = {}

    def qb_begin(qb):
        q0 = QB * qb
        x_qb = xqb_pool.tile([P, 2, E], FP16, tag="xqb", name=f"xqb_{qb}")
        nc.sync.dma_start(
            out=x_qb[:],
            in_=io["xnat"][q0:q0 + QB, :].rearrange("(s p) e -> p s e", p=P),
        )
        st[qb] = dict(
            x_qb=x_qb,
            Wacc=[accq_pool.tile([P, NKT * QB], FP16, tag="accq", name=f"wa_{qb}_{p}")
                  for p in range(2)],
            attnT8=a8_pool.tile([P, NEC, QB], E4M3, tag="attnT8", name=f"a8_{qb}"),
            ysb=ysb_pool.tile([P, 2, E], FP16, tag="ysb", name=f"y_{qb}"),
        )

    def head_front(qb, h):
        if h == 0:
            qb_begin(qb)
        q0 = QB * qb
        hb = (h % 2) * DH
        hc = h // 2
        expT = expT_pool.tile([P, NKT * QB], FP16, tag="expT",
                              name=f"expT_{qb}_{h}")
        for half in range(2):
            sc = psum_sc.tile([P, 1024], F32, tag="sc", name=f"sc_{qb}_{h}_{half}")
            for j in range(4):
                kt = 4 * half + j
                nc.tensor.matmul(
                    sc[:, QB * j:QB * (j + 1)],
                    kT[hb:hb + DH, hc, P * kt:P * (kt + 1)],
                    qT[hb:hb + DH, hc, q0:q0 + QB],
                    start=True, stop=True,
                )
            nc.scalar.activation(
                expT[:, 1024 * half:1024 * (half + 1)], sc[:],
                AF.Exp, scale=0.125,
            )
        return expT

    def tail_a(qb, h, expT):
        s = st[qb]
        hb = (h % 2) * DH
        hc = h // 2
        av = psum_av.tile([P, 512], F32, tag="av", name=f"av_{qb}_{h}")
        for kt in range(NKT):
            nc.tensor.matmul(
                av[0:DH + 1, 0:QB],
                v_sb[:, VS * kt + (DH + 1) * h:VS * kt + (DH + 1) * (h + 1)],
                expT[:, QB * kt:QB * (kt + 1)],
                start=(kt == 0), stop=(kt == NKT - 1),
            )
        inv = small.tile([1, QB], FP16, tag="inv", name=f"inv_{qb}_{h}")
        nc.vector.reciprocal(inv[:], av[DH:DH + 1, 0:QB])
        inv_bc = invbc_pool.tile([P, QB], FP16, tag="invbc", name=f"ib_{qb}_{h}")
        nc.gpsimd.partition_broadcast(inv_bc[:], inv[:])
        nc.vector.tensor_tensor(
            out=s["attnT8"][hb:hb + DH, hc, :],
            in0=av[0:DH, 0:QB], in1=inv_bc[0:DH, :], op=OP.mult,
        )
        return inv_bc

    def tail_b(qb, h, expT, inv_bc):
        s = st[qb]
        iap = inv_bc[:]
        bc_ap = bass.AP(tensor=iap.tensor, offset=iap.offset,
                        ap=[iap.ap[0], [0, NKT], iap.ap[1]])
        Wacc = s["Wacc"][h % 2]
        if h <= 1:
            nc.vector.tensor_tensor(
                out=Wacc[:].rearrange("p (n d) -> p n d", d=QB),
                in0=expT[:].rearrange("p (n d) -> p n d", d=QB),
                in1=bc_ap, op=OP.mult,
            )
        else:
            if h % 2 == 1:
                nc.gpsimd.tensor_tensor(
                    out=expT[:].rearrange("p (n d) -> p n d", d=QB),
                    in0=expT[:].rearrange("p (n d) -> p n d", d=QB),
                    in1=bc_ap, op=OP.mult,
                )
            else:
                nc.vector.tensor_tensor(
                    out=expT[:].rearrange("p (n d) -> p n d", d=QB),
                    in0=expT[:].rearrange("p (n d) -> p n d", d=QB),
                    in1=bc_ap, op=OP.mult,
                )
            nc.vector.tensor_tensor(out=Wacc[:], in0=Wacc[:], in1=expT[:],
                                     op=OP.add)

    def finalize_op(qb):
        s = st[qb]
        x_qb = s["x_qb"]
        attnT8 = s["attnT8"]
        for qs in range(2):
            for eb in range(2):
                po = psum_p1.tile([P, 512], F32, tag="p1", name=f"po_{qb}_{qs}_{eb}")
                for sub in range(2):
                    for pr in range(4):
                        nc.tensor.matmul(
                            po[:, 256 * sub:256 * (sub + 1)],
                            attnT8[:, 2 * pr:2 * pr + 2, P * qs:P * (qs + 1)],
                            wo8[:, 2 * pr:2 * pr + 2,
                                512 * eb + 256 * sub:512 * eb + 256 * (sub + 1)],
                            start=(pr == 0), stop=False,
                            perf_mode=DR,
                        )
                    nc.tensor.matmul(
                        po[:, 256 * sub:256 * (sub + 1)],
                        ones1[0:1, :],
                        bvo_row[:, E + 512 * eb + 256 * sub:
                                E + 512 * eb + 256 * (sub + 1)],
                        start=False, stop=True,
                    )
                nc.vector.scalar_tensor_tensor(
                    out=s["ysb"][:, qs, 512 * eb:512 * (eb + 1)],
                    in0=po[:], scalar=1.0 / 256.0,
                    in1=x_qb[:, qs, 512 * eb:512 * (eb + 1)],
                    op0=OP.mult, op1=OP.add,
                )

    def finalize_w(qb):
        s = st[qb]
        W0, W1 = s["Wacc"]
        nc.vector.tensor_tensor(out=W0[:], in0=W0[:], in1=W1[:], op=OP.add)
        Wacc = W0
        q0 = QB * qb
        # attn weights -> natural [q, k]; last qb transposes on the idle PE
        for qs in range(2):
            wnat = wnat_pool.tile([P, NKT, P], FP16, tag="wnat",
                                  name=f"wn_{qb}_{qs}")
            if qb == NQB - 1:
                tp = psum_p1.tile([P, NKT, P], FP16, tag="p1", name=f"tp_{qb}_{qs}")
                for kt in range(NKT):
                    nc.tensor.transpose(
                        tp[:, kt, :],
                        Wacc[:, QB * kt + P * qs:QB * kt + P * (qs + 1)],
                        ident[:],
                    )
                nc.vector.tensor_copy(wnat[:], tp[:])
            else:
                for kt in range(NKT):
                    nc.sync.dma_start_transpose(
                        wnat[:, kt, :],
                        Wacc[:, QB * kt + P * qs:QB * kt + P * (qs + 1)],
                    )
            nc.sync.dma_start(
                out=io["w16"][q0 + P * qs:q0 + P * (qs + 1), :], in_=wnat[:]
            )

    def finalize_ln(qb):
        # LayerNorm: batched stats + one-shot rsqrt (bit trick + 1 Newton)
        yqb = st[qb]["ysb"]
        mvs = []
        for qs in range(2):
            t = 2 * qb + qs
            stats = small.tile([P, 2, 6], F32, tag="stats", name=f"st_{t}")
            ychg = yqb[:, qs, :].rearrange("p (s f) -> p s f", f=512)
            for sg in range(2):
                nc.vector.bn_stats(out=stats[:, sg, :], in_=ychg[:, sg, :])
            mv = small.tile([P, 2], F32, tag="mv", name=f"mv_{t}")
            nc.vector.bn_aggr(out=mv[:], in_=stats[:])
            mvs.append(mv)
        ve = small.tile([P, 2], F32, tag="ve", name=f"ve_{qb}")
        for qs in range(2):
            nc.vector.tensor_scalar_add(out=ve[:, qs:qs + 1],
                                        in0=mvs[qs][:, 1:2], scalar1=LN_EPS)
        y0u = small.tile([P, 2], mybir.dt.uint32, tag="y0u", name=f"y0_{qb}")
        ou = bass.AP(tensor=one_u.tensor, offset=one_u[:].offset,
                     ap=[one_u[:].ap[0], [0, 2]])
        mu = bass.AP(tensor=magic_u.tensor, offset=magic_u[:].offset,
                     ap=[magic_u[:].ap[0], [0, 2]])
        nc.vector.tensor_tensor(out=y0u[:], in0=ve[:].bitcast(mybir.dt.uint32),
                                in1=ou, op=OP.logical_shift_right)
        nc.vector.tensor_tensor(out=y0u[:], in0=mu, in1=y0u[:], op=OP.subtract)
        y0 = y0u[:].bitcast(F32)
        rstd = small.tile([P, 2], F32, tag="rstd", name=f"rs_{qb}")
        tmp = small.tile([P, 2], F32, tag="tmp", name=f"tm_{qb}")
        nc.vector.tensor_tensor(out=tmp[:], in0=y0, in1=y0, op=OP.mult)
        nc.vector.tensor_tensor(out=tmp[:], in0=tmp[:], in1=ve[:], op=OP.mult)
        nc.vector.tensor_scalar(out=tmp[:], in0=tmp[:], scalar1=-0.5,
                                scalar2=1.5, op0=OP.mult, op1=OP.add)
        nc.vector.tensor_tensor(out=rstd[:], in0=y0, in1=tmp[:], op=OP.mult)
        for qs in range(2):
            t = 2 * qb + qs
            negmr = small.tile([P, 1], F32, tag="negmr", name=f"nm_{t}")
            nc.vector.tensor_tensor(out=negmr[:], in0=mvs[qs][:, 0:1],
                                    in1=rstd[:, qs:qs + 1], op=OP.mult)
            nc.vector.tensor_scalar_mul(out=negmr[:], in0=negmr[:], scalar1=-1.0)
            z16 = z_pool.tile([P, E], FP16, tag="z16", name=f"z_{t}")
            nc.scalar.activation(z16[:], yqb[:, qs, :], AF.Identity,
                                 bias=negmr[:], scale=rstd[:, qs:qs + 1])
            nc.vector.tensor_tensor(out=z16[:], in0=z16[:], in1=gamma_bc[:],
                                    op=OP.mult)
            nc.vector.tensor_tensor(out=z16[:], in0=z16[:], in1=beta_bc[:],
                                    op=OP.add)
            nc.sync.dma_start(out=io["y16"][P * t:P * (t + 1), :], in_=z16[:])

    # ---- fused pipeline: q-proj, then k-proj interleaved with heads ----
    for m in range(3):
        qk_proj(0, m)

    jobs = [(qb, h) for qb in range(NQB) for h in range(H)]
    N = len(jobs)
    fronts = {}
    invs = {}
    nf = 0
    na = 0
    nb = 0

    def emit_front():
        nonlocal nf
        qb, h = jobs[nf]
        fronts[nf] = head_front(qb, h)
        nf += 1

    def emit_a():
        nonlocal na
        qb, h = jobs[na]
        invs[na] = tail_a(qb, h, fronts[na])
        na += 1
        if h == H - 1:
            finalize_op(qb)
            if qb == NQB - 1:
                finalize_ln(qb)

    def emit_b():
        nonlocal nb
        qb, h = jobs[nb]
        tail_b(qb, h, fronts.pop(nb), invs.pop(nb))
        nb += 1
        if h == H - 1:
            finalize_w(qb)
        if nb >= 6 and (nb - 6) % H == 0 and 1 <= (nb - 6) // H < NQB:
            finalize_ln((nb - 6) // H - 1)

    def step():
        if nf < N:
            emit_front()
        if na < min(nf - 1, N) if nf < N else na < N:
            emit_a()
        if nb < min(na - 1, N) if na < N else nb < N:
            emit_b()

    for m in range(NEC):
        qk_proj(1, m)
        emit_front()
        emit_front()
        if m >= 1:
            emit_a()
            emit_a()
        if m >= 2:
            emit_b()
            emit_b()
        if m + 3 < NEC:
            qk_proj(0, m + 3)
    while nb < N:
        if nf < N:
            emit_front()
        if na < nf - 1 or (nf == N and na < N):
            emit_a()
            # drain: pull the final tail_a forward so out_proj deps resolve
            # before the remaining tail_b chain occupies the DVE queue
            if nf == N and na == N - 1:
                emit_a()
        if nb < na - 1 or (na == N and nb < N):
            emit_b()
